# revision 20
# baseline (speedup 1.0000x reference)
"""CapsNet dynamic-routing layer on 8 Trainium2 NeuronCores.

Sharding: tensor-parallel over num_caps_j (J=32 -> 4 per core). Every
(batch, j) pair's routing is independent, so there are no collectives:
each core computes u_hat[:, :, j_shard, :] plus 3 routing iterations and
returns v_J[:, j_shard, :].

Per-core dataflow (pipelined over two j-pairs):
  phase A: einsum u_hat = W.T @ u for j-pair 0 (PE, fp16, fp32 PSUM)
  phase B: routing for pair 0 (DVE/ACT) || einsum for pair 1 (PE/DMA)
  phase C: routing for pair 1
u_hat lives SBUF-resident in fp16 [b, jp, i, v] layout. Routing
contractions are fold trees on the vector engine (tensor_tensor caps at
2x mode; TENSOR_REDUCE runs 1x so folds beat it for the v-contraction).
b_r = u_hat . w_r with w_r the cumulative sum of v's (b starts at 0),
so the t-pass overwrites b_IJ each iteration.
"""

import sys

if "/opt/trn_rl_repo" not in sys.path:
    sys.path.insert(0, "/opt/trn_rl_repo")

import numpy as np

B, I, D, J, V = 128, 512, 256, 32, 32
NCORES = 8
JL = J // NCORES          # 4 j's per core
JV = JL * V               # 128
JP = 2                    # j's per pipeline pair
JPV = JP * V              # 64
DP = 128                  # contraction chunk (partitions)
EPS = 1e-9
IBLK = 16                 # i-block per DMA tile
NCH = 4                   # routing i-chunks
CHUNK = I // NCH          # 128 i's per routing chunk

_cache = {}


def _build_program():
    import concourse.tile as tile
    from concourse import bacc, mybir

    f16 = mybir.dt.float16
    f32 = mybir.dt.float32
    MULT = mybir.AluOpType.mult

    nc = bacc.Bacc("TRN2", target_bir_lowering=False, debug=False,
                   num_devices=NCORES)

    xa = nc.dram_tensor("xa", [DP, I, B], f16, kind="ExternalInput")
    xb = nc.dram_tensor("xb", [DP, I, B], f16, kind="ExternalInput")
    wts = [[nc.dram_tensor(f"w{c}{p}", [DP, I, JPV], f16,
                           kind="ExternalInput")
            for p in range(2)] for c in ("a", "b")]
    v2d = nc.dram_tensor("v2", [B, JV], f32, kind="ExternalOutput")

    with tile.TileContext(nc) as tc:
        from contextlib import ExitStack
        stack = ExitStack()
        upool = stack.enter_context(tc.tile_pool(name="uhat", bufs=2))
        xwpool = stack.enter_context(tc.tile_pool(name="xw", bufs=2))
        pspool = stack.enter_context(
            tc.tile_pool(name="psum", bufs=4, space="PSUM"))
        rpool = stack.enter_context(tc.tile_pool(name="rout", bufs=1))
        ppool = stack.enter_context(tc.tile_pool(name="prod", bufs=1))

        eps_t = rpool.tile([B, 1], f32, tag="eps")
        nc.gpsimd.memset(eps_t[:], EPS)

        WBLK = 32

        def emit_einsum(pair, s0_cb=None):
            """u_hat for j-pair -> U tile [B, JP, I, V] fp16.
            s0_cb(U, h) is emitted after the h-th CHUNK of i is in U, so
            the s0 fold chains run on the vector engine (idle during the
            einsum) as soon as their inputs land."""
            U = upool.tile([B, JP, I, V], f16, tag="U")
            next_h = 0
            for wblk in range(I // WBLK):
                w0 = wblk * WBLK
                wa_t = xwpool.tile([DP, WBLK, JPV], f16, tag="wa")
                nc.sync.dma_start(wa_t[:],
                                  wts[0][pair].ap()[:, w0:w0 + WBLK, :])
                wb_t = xwpool.tile([DP, WBLK, JPV], f16, tag="wb")
                nc.sync.dma_start(wb_t[:],
                                  wts[1][pair].ap()[:, w0:w0 + WBLK, :])
                for sub in range(WBLK // IBLK):
                    i0 = w0 + sub * IBLK
                    xa_t = xwpool.tile([DP, IBLK, B], f16, tag="xa")
                    nc.sync.dma_start(xa_t[:], xa.ap()[:, i0:i0 + IBLK, :])
                    xb_t = xwpool.tile([DP, IBLK, B], f16, tag="xb")
                    nc.sync.dma_start(xb_t[:], xb.ap()[:, i0:i0 + IBLK, :])
                    for g in range(IBLK // 8):
                        ps = pspool.tile([B, 8, JPV], f32)  # one 2KB bank
                        for k in range(8):
                            il = g * 8 + k
                            iw = sub * IBLK + il
                            nc.tensor.matmul(
                                ps[:, k, :], xa_t[:, il, :], wa_t[:, iw, :],
                                start=True, stop=False)
                            nc.tensor.matmul(
                                ps[:, k, :], xb_t[:, il, :], wb_t[:, iw, :],
                                start=False, stop=True)
                        ia = i0 + g * 8
                        # PSUM [b, i8, (j2 v32)] -> U[b, j, ia:ia+8, v]
                        nc.scalar.copy(
                            U[:, :, ia:ia + 8, :],
                            ps.rearrange("p i (j v) -> p j i v", j=JP))
                if s0_cb is not None:
                    done_i = w0 + WBLK
                    while (next_h + 1) * CHUNK <= done_i:
                        s0_cb(U, next_h)
                        next_h += 1
            return U

        def fold_i(prod, out_ap, accumulate, eng=None, tagp=""):
            """Fold [B, CHUNK, V] fp16 over the i axis down to [B, V],
            then copy/add into out_ap ([B, V], fp32)."""
            if eng is None:
                eng = nc.vector
            cur = prod
            n = CHUNK
            while n > 2:
                nh = n // 2
                nxt = ppool.tile([B, nh * V], f16, tag=f"{tagp}fi{nh}")
                eng.tensor_add(
                    nxt[:].rearrange("p (i v) -> p i v", v=V),
                    cur[:, 0:nh, :], cur[:, nh:n, :])
                cur = nxt[:].rearrange("p (i v) -> p i v", v=V)
                n = nh
            if accumulate:
                tmp = ppool.tile([B, V], f16, tag=f"{tagp}fi1")
                eng.tensor_add(tmp[:], cur[:, 0, :], cur[:, 1, :])
                eng.tensor_add(out_ap, out_ap, tmp[:])
            else:
                eng.tensor_add(out_ap, cur[:, 0, :], cur[:, 1, :])

        def fold_v(prod, out_ap):
            """Fold [B, CHUNK, V] fp16 over the v axis -> out_ap
            ([B, CHUNK] fp32 slice of b_IJ)."""
            cur = prod
            n = V
            while n > 2:
                nh = n // 2
                nxt = ppool.tile([B, CHUNK * nh], f16, tag=f"fv{nh}")
                nc.vector.tensor_add(
                    nxt[:].rearrange("p (i v) -> p i v", v=nh),
                    cur[:, :, 0:nh], cur[:, :, nh:n])
                cur = nxt[:].rearrange("p (i v) -> p i v", v=nh)
                n = nh
            nc.vector.tensor_add(out_ap, cur[:, :, 0], cur[:, :, 1])

        def squash(s_ap, v_ap, sq, n2, d1, r1, rt, r2, fac):
            # v = s * n2/(1+n2)/sqrt(n2+EPS), per (b, j) over v-axis
            nc.vector.tensor_mul(sq[:], s_ap, s_ap)
            nc.vector.reduce_sum(n2[:], sq[:], axis=mybir.AxisListType.X)
            nc.scalar.add(d1[:], n2[:], 1.0)
            nc.vector.reciprocal(r1[:], d1[:])
            nc.scalar.activation(rt[:], n2[:],
                                 mybir.ActivationFunctionType.Sqrt,
                                 bias=eps_t[:])
            nc.vector.reciprocal(r2[:], rt[:])
            nc.vector.tensor_mul(fac[:], n2[:], r1[:])
            nc.vector.tensor_mul(fac[:], fac[:], r2[:])
            fb = fac[:].unsqueeze(2).broadcast_to([B, JP, V])
            nc.vector.tensor_tensor(v_ap, s_ap, fb, op=MULT)

        def make_s0_tiles(pair):
            s_acc = rpool.tile([B, JP, V], f32, tag=f"s_acc{pair}")
            return s_acc

        def emit_routing(pair, U, s_acc):
            w16 = rpool.tile([B, JP, V], f16, tag="w16")
            bij = rpool.tile([B, JP, I], f32, tag="bij")
            e16 = rpool.tile([B, JP, I], f16, tag="e16")
            c16 = rpool.tile([B, JP, I], f16, tag="c16")
            Ssum = rpool.tile([B, JP], f32, tag="Ssum")
            Srec = rpool.tile([B, JP], f32, tag="Srec")
            cfac = rpool.tile([B, JP], f16, tag="cfac")
            sq = rpool.tile([B, JP, V], f32, tag="sq")
            n2 = rpool.tile([B, JP], f32, tag="n2")
            d1 = rpool.tile([B, JP], f32, tag="d1")
            r1 = rpool.tile([B, JP], f32, tag="r1")
            rt = rpool.tile([B, JP], f32, tag="rt")
            r2 = rpool.tile([B, JP], f32, tag="r2")
            fac = rpool.tile([B, JP], f32, tag="fac")
            vout = rpool.tile([B, JP, V], f32, tag="vout")

            # s0 already accumulated into s_acc during the einsum
            squash(s_acc[:], vout[:], sq, n2, d1, r1, rt, r2, fac)
            nc.vector.tensor_copy(w16[:], vout[:])      # w = v0

            for r in (1, 2):
                # t-pass: bij[b,j,i] = sum_v U*w16   (b_r = U . w_r)
                for j in range(JP):
                    for h in range(NCH):
                        isl = slice(h * CHUNK, (h + 1) * CHUNK)
                        prod = ppool.tile([B, CHUNK, V], f16, tag="prod")
                        wb_ = (w16[:, j, :].unsqueeze(1)
                               .broadcast_to([B, CHUNK, V]))
                        nc.vector.tensor_tensor(
                            prod[:], U[:, j, isl, :], wb_, op=MULT)
                        fold_v(prod, bij[:, j, isl])

                # softmax over i (|b| <= ~3, skip max subtraction)
                nc.scalar.activation(e16[:], bij[:],
                                     mybir.ActivationFunctionType.Exp)
                nc.vector.reduce_sum(Ssum[:], e16[:],
                                     axis=mybir.AxisListType.X)
                nc.vector.reciprocal(Srec[:], Ssum[:])
                nc.scalar.mul(cfac[:], Srec[:], float(I))
                cb = cfac[:].unsqueeze(2).broadcast_to([B, JP, I])
                nc.vector.tensor_tensor(c16[:], e16[:], cb, op=MULT)

                # s-pass: s[b,j,v] = sum_i c16*U
                for j in range(JP):
                    for h in range(NCH):
                        isl = slice(h * CHUNK, (h + 1) * CHUNK)
                        prod = ppool.tile([B, CHUNK, V], f16, tag="prod")
                        cb_ = (c16[:, j, isl].unsqueeze(2)
                               .broadcast_to([B, CHUNK, V]))
                        nc.vector.tensor_tensor(
                            prod[:], U[:, j, isl, :], cb_, op=MULT)
                        fold_i(prod, s_acc[:, j, :], accumulate=h > 0)
                squash(s_acc[:], vout[:], sq, n2, d1, r1, rt, r2, fac)
                if r == 1:
                    nc.vector.tensor_add(w16[:], w16[:], vout[:])
                else:
                    jv0 = pair * JPV
                    nc.sync.dma_start(
                        v2d.ap()[:, jv0:jv0 + JPV],
                        vout[:].rearrange("p j v -> p (j v)"))

        def s0_cb_for(s_acc, eng, tagp):
            # pair 0's s0 folds run on the (idle) vector engine during
            # phase A; pair 1's run on gpsimd so they don't steal DVE
            # time from pair 0's routing in phase B.
            def cb(U, h):
                isl = slice(h * CHUNK, (h + 1) * CHUNK)
                for j in range(JP):
                    fold_i(U[:, j, isl, :], s_acc[:, j, :],
                           accumulate=h > 0, eng=eng, tagp=tagp)
            return cb

        s_acc0 = make_s0_tiles(0)
        U0 = emit_einsum(0, s0_cb=s0_cb_for(s_acc0, nc.vector, ""))
        emit_routing(0, U0, s_acc0)
        s_acc1 = make_s0_tiles(1)
        U1 = emit_einsum(1, s0_cb=s0_cb_for(s_acc1, nc.gpsimd, "g"))
        emit_routing(1, U1, s_acc1)
        stack.close()

    nc.compile()
    return nc


def _get_program():
    if "nc" not in _cache:
        _cache["nc"] = _build_program()
    return _cache["nc"]


def _prep_inputs(x, W):
    """Host-side shard + transpose + fp16 cast."""
    u = np.ascontiguousarray(x[..., 0])                   # [B, I, D] f32
    xt = np.ascontiguousarray(u.transpose(2, 1, 0)).astype(np.float16)
    xa_np = np.ascontiguousarray(xt[:DP])                 # [128, I, B]
    xb_np = np.ascontiguousarray(xt[DP:])
    W0 = W[0]                                             # [I, J, D, V]
    in_maps = []
    for c in range(NCORES):
        Wc = W0[:, c * JL:(c + 1) * JL]                   # [I, JL, D, V]
        Wt = Wc.transpose(2, 0, 1, 3)                     # [D, I, JL, V]
        Wt = Wt.reshape(D, I, JV).astype(np.float16)
        in_maps.append({
            "xa": xa_np,
            "xb": xb_np,
            "wa0": np.ascontiguousarray(Wt[:DP, :, :JPV]),
            "wa1": np.ascontiguousarray(Wt[:DP, :, JPV:]),
            "wb0": np.ascontiguousarray(Wt[DP:, :, :JPV]),
            "wb1": np.ascontiguousarray(Wt[DP:, :, JPV:]),
        })
    return in_maps


def run_cores(x, W, trace=False):
    from concourse import bass_utils
    nc = _get_program()
    in_maps = _prep_inputs(x, W)
    res = bass_utils.run_bass_kernel_spmd(
        nc, in_maps, core_ids=list(range(NCORES)), trace=trace)
    return res


def kernel(x, W):
    x = np.asarray(x)
    W = np.asarray(W)
    res = run_cores(x, W, trace=False)
    out = np.empty((B, J, V, 1), dtype=np.float32)
    for c in range(NCORES):
        vc = res.results[c]["v2"].reshape(B, JL, V)
        out[:, c * JL:(c + 1) * JL, :, 0] = vc
    return out
